# revision 1
# baseline (speedup 1.0000x reference)
"""Trainium2 Bass kernel for nn_ContrastiveLoss (8-core data-parallel).

Contract: kernel(**inputs) takes the FULL unsharded inputs
(feats1 [2048,512] f32, feats2 [2048,512] f32, overlap_inds [8] i32, bs=256)
and returns the full output (acc, loss) like the reference.

Math restructuring (see reference):
  feats = concat(feats1, feats2)  [N=4096, F=512]
  G = feats @ feats.T ; sim = exp(TEMP*G) ; log(sim) = TEMP*G
  labels are constant on 16 blocks of 256 consecutive rows, so every mask
  (same / pos / neg / cross) is block-constant (minus the diagonal).  Each
  label appears in at most two blocks (one per half), so each row has one
  "self" positive block and at most one "partner" positive block.

  Per row i:   negsum_i = sum_{neg blocks} rowsum(e)
               thr_i    = max_{neg blocks} rowmax(e)
               count_i  = #{pos j : e_ij > thr_i}   (acc numerator)
               lossnum_i = PW_i*log(negsum_i) - TEMP*sum_{pos} cross*G_ij

Device computes, per core (rows of 2 groups), streaming over column tiles:
  e = exp(TEMP*G) with fused per-256-block row-sums (ScalarE accum_out) and
  per-block row-max (VectorE).  The first 512 permuted columns (= the two
  candidate positive blocks) of e are written out ("pose").  Everything else
  is assembled on the host from the tiny per-block stats; borderline count
  rows are refined exactly on the host from feats.

Sharding: core c owns row groups {c, 8+c}.  Host hands each core featsT with
columns permuted to [block c, block 8+c, remaining 14 blocks], so one uniform
SPMD NEFF serves all cores (lhsT = first 512 permuted columns; pose = first
512 columns of each row band).  Inputs are replicated (8 MB/core) -> no
collectives; scalar reduction happens on the host.
"""

import os
import sys

sys.path.insert(0, "/opt/trn_rl_repo")
# this container has no NTFF trace hook (antenv is a stub); make sure a
# stray BASS_TRACE env can never route us onto that path
os.environ["BASS_NEVER_TRACE"] = "1"

from contextlib import ExitStack

import numpy as np

import concourse.mybir as mybir
import concourse.tile as tile
from concourse import bacc
from concourse.bass_utils import run_bass_kernel_spmd

TEMP = 0.02
OTHERWEIGHT = 0.5

NCORES = 8
N = 4096          # total rows (feats1 + feats2)
F = 512           # feature dim
BS = 256          # rows per group/block
NBLK = 16         # 256-row blocks
ROWS_PER_CORE = 512
MTILES = 4        # 128-row subtiles per core
NTILES = 8        # 512-col tiles per row band
KT = 4            # 128-row contraction tiles of F

_BUILT = None     # cached (nc,) build
_LAST_RESULTS = None


def _build_nc():
    """Build the uniform SPMD Tile kernel (one NEFF for all 8 cores)."""
    f32 = mybir.dt.float32
    f32r = mybir.dt.float32r

    nc = bacc.Bacc("TRN2", target_bir_lowering=False, debug=False)
    ft_d = nc.dram_tensor("ft", [F, N], f32r, kind="ExternalInput")
    pose_d = nc.dram_tensor("pose", [MTILES, 128, 512], f32, kind="ExternalOutput")
    # stats per m-subtile (20 cols): [0:2] per-block e-sums of tile 0,
    # [2:9] pair e-sums of tiles 1..7, [10:12] per-block e-maxes of tile 0,
    # [12:19] pair e-maxes of tiles 1..7.  Tiles 1..7 are all-negative for
    # every row group (the permutation puts both positive candidates in
    # tile 0), so pair granularity suffices there.
    stat_d = nc.dram_tensor("stat", [128, MTILES * 20], f32, kind="ExternalOutput")

    Exp = mybir.ActivationFunctionType.Exp

    with tile.TileContext(nc) as tc, ExitStack() as ctx:
        ftp = ctx.enter_context(tc.tile_pool(name="ft", bufs=1))
        posp = ctx.enter_context(tc.tile_pool(name="pose", bufs=1))
        ep = ctx.enter_context(tc.tile_pool(name="e", bufs=12))
        dp = ctx.enter_context(tc.tile_pool(name="dummy", bufs=2))
        statp = ctx.enter_context(tc.tile_pool(name="stat", bufs=1))
        psp = ctx.enter_context(tc.tile_pool(name="ps", bufs=8, space="PSUM"))

        ft_t = [ftp.tile([128, N], f32r, name=f"ft{kf}", tag=f"ft{kf}") for kf in range(KT)]
        # stream the input in 512-col chunks (n-tile granularity) so PE can
        # start as soon as the first MB lands
        for q in range(NTILES):
            for kf in range(KT):
                nc.sync.dma_start(
                    ft_t[kf][:, q * 512 : (q + 1) * 512],
                    ft_d.ap()[kf * 128 : (kf + 1) * 128, q * 512 : (q + 1) * 512],
                )

        pose_t = [posp.tile([128, 512], f32, name=f"pose{m}", tag=f"pose{m}") for m in range(MTILES)]
        stat_t = statp.tile([128, MTILES * 20], f32, tag="stat")

        for n in range(NTILES):
            for m in range(MTILES):
                ps = psp.tile([128, 512], f32, name="ps", tag="ps")
                for kf in range(KT):
                    nc.tensor.matmul(
                        ps[:],
                        ft_t[kf][:, m * 128 : (m + 1) * 128],
                        ft_t[kf][:, n * 512 : (n + 1) * 512],
                        start=(kf == 0),
                        stop=(kf == KT - 1),
                    )
                base = m * 20
                et = pose_t[m] if n == 0 else ep.tile([128, 512], f32, name="et", tag="e")
                if n == 0:
                    # tile 0: per-block sums (2 fused exp+accum) + per-block max
                    for h in range(2):
                        nc.scalar.activation(
                            et[:, h * 256 : (h + 1) * 256],
                            ps[:, h * 256 : (h + 1) * 256],
                            Exp,
                            scale=TEMP,
                            accum_out=stat_t[:, base + h : base + h + 1],
                        )
                    nc.vector.tensor_reduce(
                        stat_t[:, base + 10 : base + 12],
                        et[:].rearrange("p (b x) -> p b x", b=2),
                        axis=mybir.AxisListType.X,
                        op=mybir.AluOpType.max,
                    )
                    nc.sync.dma_start(pose_d.ap()[m], pose_t[m][:])
                else:
                    # tiles 1..7: fused exp+pair-sum on ACT, pair-max on DVE
                    nc.scalar.activation(
                        et[:],
                        ps[:],
                        Exp,
                        scale=TEMP,
                        accum_out=stat_t[:, base + 1 + n : base + 2 + n],
                    )
                    nc.vector.tensor_reduce(
                        stat_t[:, base + 11 + n : base + 12 + n],
                        et[:],
                        axis=mybir.AxisListType.X,
                        op=mybir.AluOpType.max,
                    )

        # two half-stores: m=0,1 stats complete two banks before m=2,3 at
        # n=7, so the first store overlaps the last banks' compute
        nc.sync.dma_start(stat_d.ap()[:, 0:40], stat_t[:, 0:40])
        nc.sync.dma_start(stat_d.ap()[:, 40:80], stat_t[:, 40:80])

    nc.compile()
    return nc


def _labels_np(ov, bs):
    K = ov.shape[0]
    labels1 = np.repeat(np.arange(K), bs)
    non = (ov == 0).astype(np.int64)
    excl = np.cumsum(non) - non
    cls2 = np.where(ov.astype(bool), np.arange(K), K + excl)
    labels2 = np.repeat(cls2, bs)
    return np.concatenate([labels1, labels2])


def kernel(feats1, feats2, overlap_inds, bs):
    global _BUILT, _LAST_RESULTS
    bs = int(bs)
    feats1 = np.asarray(feats1, np.float32)
    feats2 = np.asarray(feats2, np.float32)
    ov = np.asarray(overlap_inds)
    assert feats1.shape == (2048, 512) and feats2.shape == (2048, 512)
    assert bs == BS and ov.shape == (8,)

    feats = np.concatenate([feats1, feats2])              # [N, F]
    featsT = np.ascontiguousarray(feats.T)                # [F, N]
    labels = _labels_np(ov, bs)                           # [N]
    lblock = labels[::BS]                                 # [16] per-block label

    # per-core permuted inputs: blocks [c, 8+c, rest]
    perms = []
    in_maps = []
    for c in range(NCORES):
        pb = [c, 8 + c] + [b for b in range(NBLK) if b not in (c, 8 + c)]
        perms.append(pb)
        cols = np.concatenate([np.arange(b * BS, (b + 1) * BS) for b in pb])
        in_maps.append({"ft": np.ascontiguousarray(featsT[:, cols])})

    if _BUILT is None:
        _BUILT = _build_nc()
    nc = _BUILT

    try:
        res = run_bass_kernel_spmd(nc, in_maps, core_ids=list(range(NCORES)))
    except Exception:
        # transient NRT/device hiccups have been observed on this fabric;
        # one clean retry is cheap insurance
        res = run_bass_kernel_spmd(nc, in_maps, core_ids=list(range(NCORES)))
    _LAST_RESULTS = res

    # ---- host assembly ----
    counts = np.bincount(labels)
    total_pos = float((counts[labels] - 1).sum())

    cnt_rows = np.zeros(N, np.float64)
    lossnum_rows = np.zeros(N, np.float64)
    need_refine = []

    lanes = np.arange(128)
    for c in range(NCORES):
        out = res.results[c]
        pose = out["pose"]                       # [4, 128, 512] f32 (e-domain)
        stat = out["stat"]                       # [128, 128]
        for m in range(MTILES):
            b_self = c if m < 2 else 8 + c
            slot_self = 0 if m < 2 else 1
            b_part = 8 + c if m < 2 else c
            paired = lblock[b_self] == lblock[b_part]

            base = m * 20
            pair_sums = stat[:, base + 2 : base + 9].astype(np.float64)  # [128,7]
            pair_maxs = stat[:, base + 12 : base + 19]                   # [128,7]
            negsum = pair_sums.sum(axis=1)                               # [128]
            thr = pair_maxs.max(axis=1)                                  # [128] f32
            if not paired:  # sibling candidate block is a negative
                negsum = negsum + stat[:, base + (1 - slot_self)].astype(np.float64)
                thr = np.maximum(thr, stat[:, base + 10 + (1 - slot_self)])

            pm = pose[m]                                            # [128, 512]
            sl_self = slice(slot_self * 256, slot_self * 256 + 256)
            sl_part = slice((1 - slot_self) * 256, (1 - slot_self) * 256 + 256)
            diag_col = slot_self * 256 + (m % 2) * 128 + lanes
            e_diag = pm[lanes, diag_col]

            cnt = (pm[:, sl_self] > thr[:, None]).sum(axis=1).astype(np.float64)
            cnt -= (e_diag > thr)
            if paired:
                cnt += (pm[:, sl_part] > thr[:, None]).sum(axis=1)

            g = np.log(pm.astype(np.float64)) / TEMP
            g_diag = g[lanes, diag_col]
            possum = g[:, sl_self].sum(axis=1) - g_diag
            pw = 255.0
            if paired:
                possum += OTHERWEIGHT * g[:, sl_part].sum(axis=1)
                pw += OTHERWEIGHT * 256.0
            lossnum = pw * np.log(negsum) - TEMP * possum

            rows = b_self * BS + (m % 2) * 128 + lanes
            cnt_rows[rows] = cnt
            lossnum_rows[rows] = lossnum

            # borderline rows -> exact host recount (matmul-precision guard)
            thr_g = np.log(thr.astype(np.float64)) / TEMP
            marg = np.abs(g[:, sl_self] - thr_g[:, None])
            marg[lanes, (m % 2) * 128 + lanes] = np.inf  # diagonal isn't pos
            mmin = marg.min(axis=1)
            if paired:
                mmin = np.minimum(mmin, np.abs(g[:, sl_part] - thr_g[:, None]).min(axis=1))
            # also guard the diagonal comparison we subtracted
            mmin = np.minimum(mmin, np.abs(g_diag - thr_g))
            for p in np.nonzero(mmin < 0.25)[0]:
                need_refine.append(rows[p])

    # exact recount of borderline rows, replicating the reference ops
    for i in set(need_refine):
        g_row = feats[i] @ feats.T                       # f32
        sim = np.exp((g_row * np.float32(TEMP)).astype(np.float32))
        negm = labels != labels[i]
        mneg = sim[negm].max()
        posm = labels == labels[i]
        posm[i] = False
        cnt_rows[i] = float((sim[posm] > mneg).sum())

    acc = np.float32(cnt_rows.sum() / total_pos)
    loss = np.float32(lossnum_rows.sum() / total_pos)
    return acc, loss



# revision 2
# speedup vs baseline: 1.0181x; 1.0181x over previous
"""Trainium2 Bass kernel for nn_ContrastiveLoss (8-core data-parallel).

fp8e4 DoubleRow matmuls (0.5 cyc/row), n-major pipeline:
  ft SBUF [128, 16384] fp8e4: col = k*8192 + q*1024 + s*512 + x,
      value = feats[colperm[q*512+x], k*256 + s*128 + p]
  Per col-tile n (512 cols x 4 m-subtiles): 8 DoubleRow matmuls
      -> PSUM [128, 2048] (ping-pong over n parity); PE p-state warmup MMs.
  Every tile: ACT exp(TEMP*G) -> bf16 e-tile.
  Tiles 1+2, 3+4: DVE pairwise add/max trees + per-m closers -> stats.
  Tiles 5, 6, 0(=both positive-candidate blocks), 7: shipped raw (bf16 e).
Host: negsum/thr/count/margins/possum(exact blocksum trick)/loss assembly;
borderline rows (|G - thr| < MARGIN) recounted exactly in f32.
"""

import os
import sys

sys.path.insert(0, "/opt/trn_rl_repo")
os.environ["BASS_NEVER_TRACE"] = "1"

from contextlib import ExitStack

import numpy as np
import ml_dtypes

import concourse.mybir as mybir
import concourse.tile as tile
from concourse import bacc
from concourse.bass_utils import run_bass_kernel_spmd

TEMP = 0.02
OTHERWEIGHT = 0.5

NCORES = 8
N = 4096
F = 512
BS = 256
NBLK = 16
MTILES = 4
NT = 8            # 512-col tiles
MARGIN = 8.0      # G-domain refinement margin (fp8 matmul noise ~1.2 std)
WARM_MMS = 55     # PE p-state warmup matmuls

F8 = ml_dtypes.float8_e4m3
BF16 = ml_dtypes.bfloat16

_BUILT = None
_LAST_RESULTS = None


def _build_nc():
    f32 = mybir.dt.float32
    bf16 = mybir.dt.bfloat16
    fp8 = mybir.dt.float8e4
    Exp = mybir.ActivationFunctionType.Exp
    DR = mybir.MatmulPerfMode.DoubleRow
    X = mybir.AxisListType.X

    nc = bacc.Bacc("TRN2", target_bir_lowering=False, debug=False)
    ft_d = nc.dram_tensor("ft", [NT, 128, 2048], fp8, kind="ExternalInput")
    # shipped e-tiles (e-domain bf16): slots [n5, n6, n0(tile0), n7]
    eship_d = nc.dram_tensor("eship", [4, 128, 2048], bf16, kind="ExternalOutput")
    # pair stats: [0:4]=sum12 [4:8]=max12 [8:12]=sum34 [12:16]=max34
    stat_d = nc.dram_tensor("stat", [128, 16], bf16, kind="ExternalOutput")

    ADD = mybir.AluOpType.add
    MAX = mybir.AluOpType.max

    with tile.TileContext(nc) as tc, ExitStack() as ctx:
        ftp = ctx.enter_context(tc.tile_pool(name="ft", bufs=1))
        ep = ctx.enter_context(tc.tile_pool(name="e", bufs=7))
        trp = ctx.enter_context(tc.tile_pool(name="tr", bufs=1))
        statp = ctx.enter_context(tc.tile_pool(name="stat", bufs=1))
        psp = ctx.enter_context(tc.tile_pool(name="ps", bufs=1, space="PSUM"))

        ft_t = ftp.tile([128, 16384], fp8, name="ft", tag="ft")
        # view [p, k, q, s, x]
        ftr = ft_t[:].rearrange("p (k q s x) -> p k q s x", k=2, q=NT, s=2)
        ftkr = ft_t[:].rearrange("p (k r) -> p k r", k=2)
        for q in range(NT):
            # one DMA per col-tile: dst [p, k, (s x)=1024], src contiguous
            nc.sync.dma_start(
                ftkr[:, :, q * 1024:(q + 1) * 1024],
                ft_d.ap()[q],
            )

        stat_t = statp.tile([128, 16], bf16, tag="stat")
        # pair partials + halving scratch
        s12 = trp.tile([128, 2048], bf16, name="s12", tag="s12")
        m12 = trp.tile([128, 2048], bf16, name="m12", tag="m12")
        s34 = trp.tile([128, 2048], bf16, name="s34", tag="s34")
        m34 = trp.tile([128, 2048], bf16, name="m34", tag="m34")


        ps_t = [psp.tile([128, 2048], f32, name=f"ps{i}", tag=f"ps{i}") for i in range(2)]

        # PE p-state warmup: keep PE busy from t~0 so real matmuls hit full
        # clock. Warm MMs write a scratch psum region later overwritten.
        warm = ftp.tile([128, 256], fp8, name="warm", tag="warm")
        nc.vector.memset(warm[:], 0.0)
        wr = warm[:].rearrange("p (s x) -> p s x", s=2)
        for _ in range(WARM_MMS):
            nc.tensor.matmul(
                ps_t[0][0:128, 0:128], wr[:, :, 0:128], wr[:, :, 0:128],
                start=True, stop=True, perf_mode=DR,
            )

        def closers(pair_t, col0, op, tag):
            # [128,(4m)(512x)] -> halve twice on DVE 2x -> short 1x reduce
            h1 = trp.tile([128, 1024], bf16, name=f"h1{tag}", tag=f"h1{tag}")
            h2 = trp.tile([128, 512], bf16, name=f"h2{tag}", tag=f"h2{tag}")
            pr = pair_t[:].rearrange("p (m x) -> p m x", m=MTILES)
            ha = h1[:].rearrange("p (m x) -> p m x", m=MTILES)
            hb = h2[:].rearrange("p (m x) -> p m x", m=MTILES)
            with nc.allow_low_precision(reason="bf16 partials; error averages out"):
                nc.vector.tensor_tensor(ha, pr[:, :, 0:256], pr[:, :, 256:512], op=op)
                nc.vector.tensor_tensor(hb, ha[:, :, 0:128], ha[:, :, 128:256], op=op)
                nc.vector.tensor_reduce(
                    stat_t[:, col0:col0 + 4], hb, axis=mybir.AxisListType.X, op=op)

        order = [1, 2, 3, 4, 5, 6, 0, 7]
        ship_slot = {5: 0, 6: 1, 0: 2, 7: 3}
        et_n = {}
        for pos, n in enumerate(order):
            ps = ps_t[pos % 2]
            for m, k in [(m, k) for m in range(MTILES) for k in range(2)]:
                nc.tensor.matmul(
                    ps[:, m * 512:(m + 1) * 512],
                    ftr[:, k, 0, :, m * 128:(m + 1) * 128],   # lhsT [128,2,128]
                    ftr[:, k, n, :, :],                       # rhs  [128,2,512]
                    start=(k == 0),
                    stop=(k == 1),
                    perf_mode=DR,
                )
            et = ep.tile([128, 2048], bf16, name="et", tag="e")
            if n == 7:
                # split the last exp + ship per half: shorter DMA tail
                nc.scalar.activation(et[:, 0:1024], ps[:, 0:1024], Exp, scale=TEMP)
                nc.sync.dma_start(eship_d.ap()[3, :, 0:1024], et[:, 0:1024])
                nc.scalar.activation(et[:, 1024:2048], ps[:, 1024:2048], Exp, scale=TEMP)
                nc.sync.dma_start(eship_d.ap()[3, :, 1024:2048], et[:, 1024:2048])
            else:
                nc.scalar.activation(et[:], ps[:], Exp, scale=TEMP)
            et_n[n] = et
            with nc.allow_low_precision(reason="bf16 tree; error averages out"):
                if n == 2:
                    nc.vector.tensor_tensor(s12[:], et_n[1][:], et[:], op=ADD)
                    nc.vector.tensor_tensor(m12[:], et_n[1][:], et[:], op=MAX)
                    closers(s12, 0, ADD, "s12")
                    closers(m12, 4, MAX, "m12")
                elif n == 4:
                    nc.vector.tensor_tensor(s34[:], et_n[3][:], et[:], op=ADD)
                    nc.vector.tensor_tensor(m34[:], et_n[3][:], et[:], op=MAX)
                    closers(s34, 8, ADD, "s34")
                    closers(m34, 12, MAX, "m34")
                    nc.gpsimd.dma_start(stat_d.ap()[:], stat_t[:])
            if n in (5, 6, 0):
                nc.sync.dma_start(eship_d.ap()[ship_slot[n]], et[:])

    nc.compile()
    return nc


def _labels_np(ov, bs):
    K = ov.shape[0]
    labels1 = np.repeat(np.arange(K), bs)
    non = (ov == 0).astype(np.int64)
    excl = np.cumsum(non) - non
    cls2 = np.where(ov.astype(bool), np.arange(K), K + excl)
    labels2 = np.repeat(cls2, bs)
    return np.concatenate([labels1, labels2])


def kernel(feats1, feats2, overlap_inds, bs):
    global _BUILT, _LAST_RESULTS
    bs = int(bs)
    feats1 = np.asarray(feats1, np.float32)
    feats2 = np.asarray(feats2, np.float32)
    ov = np.asarray(overlap_inds)
    assert feats1.shape == (2048, 512) and feats2.shape == (2048, 512)
    assert bs == BS and ov.shape == (8,)

    feats = np.concatenate([feats1, feats2])              # [N, F]
    labels = _labels_np(ov, bs)                           # [N]
    lblock = labels[::BS]                                 # [16]

    f8 = feats.astype(F8)                                 # fp8-quantized

    in_maps = []
    perms = []
    for c in range(NCORES):
        pb = [c, 8 + c] + [b for b in range(NBLK) if b not in (c, 8 + c)]
        perms.append(pb)
        cols = np.concatenate([np.arange(b * BS, (b + 1) * BS) for b in pb])
        fq = f8[cols]                                     # [4096, 512]
        # [q, x, k, s, p] -> [q, p, k, s, x] -> [8, 128, 2048]
        arr = fq.reshape(NT, 512, 2, 2, 128).transpose(0, 4, 2, 3, 1)
        arr = np.ascontiguousarray(arr.reshape(NT, 128, 2048))
        in_maps.append({"ft": arr})

    if _BUILT is None:
        _BUILT = _build_nc()
    nc = _BUILT

    try:
        res = run_bass_kernel_spmd(nc, in_maps, core_ids=list(range(NCORES)))
    except Exception:
        res = run_bass_kernel_spmd(nc, in_maps, core_ids=list(range(NCORES)))
    _LAST_RESULTS = res

    # ---- host assembly ----
    counts = np.bincount(labels)
    total_pos = float((counts[labels] - 1).sum())

    # exact possum via block sums (f64)
    feats64 = feats.astype(np.float64)
    Sblk = feats64.reshape(NBLK, BS, F).sum(axis=1)       # [16, F]
    gblk = feats64 @ Sblk.T                               # [N, 16] exact
    gdiag = (feats64 * feats64).sum(axis=1)               # [N]

    cnt_rows = np.zeros(N, np.float64)
    lossnum_rows = np.zeros(N, np.float64)
    need_refine = []

    lanes = np.arange(128)
    for c in range(NCORES):
        out = res.results[c]
        stat = out["stat"].astype(np.float64)             # [128, 16] e-domain
        eship = out["eship"].astype(np.float64)           # [4, 128, 2048] e-domain
        pose_e = eship[2]                                 # tile0, e-domain
        with np.errstate(divide="ignore"):
            pose = np.log(pose_e) / TEMP                  # [128, 2048] G-domain
        esh = eship[[0, 1, 3]].reshape(3, 128, 4, 512)    # [ship, p, m, x]
        negsum_big = (stat[:, 0:4] + stat[:, 8:12]
                      + esh.sum(axis=(0, 3)))             # [128, 4]
        bigmax = np.maximum(np.maximum(stat[:, 4:8], stat[:, 12:16]),
                            esh.max(axis=3).max(axis=0))

        for m in range(MTILES):
            b_self = c if m < 2 else 8 + c
            b_part = 8 + c if m < 2 else c
            slot_self = 0 if m < 2 else 1
            paired = lblock[b_self] == lblock[b_part]

            g0 = pose[:, m * 512:(m + 1) * 512]           # [128, 512] tile0
            sl_self = slice(slot_self * 256, slot_self * 256 + 256)
            sl_part = slice((1 - slot_self) * 256, (1 - slot_self) * 256 + 256)
            g_self = g0[:, sl_self]                       # [128, 256]
            g_part = g0[:, sl_part]

            diag_in_self = (m % 2) * 128 + lanes          # col within self block
            e_part = np.exp(TEMP * g_part)

            negsum = negsum_big[:, m].copy()
            thr_e = bigmax[:, m].copy()
            if not paired:
                negsum = negsum + e_part.sum(axis=1)
                thr_e = np.maximum(thr_e, e_part.max(axis=1))
            thr_g = np.log(thr_e) / TEMP

            mask_self = np.ones((128, 256), bool)
            mask_self[lanes, diag_in_self] = False
            cnt = ((g_self > thr_g[:, None]) & mask_self).sum(axis=1).astype(np.float64)
            marg = np.abs(g_self - thr_g[:, None])
            marg[lanes, diag_in_self] = np.inf
            mmin = marg.min(axis=1)
            if paired:
                cnt += (g_part > thr_g[:, None]).sum(axis=1)
                mmin = np.minimum(mmin, np.abs(g_part - thr_g[:, None]).min(axis=1))

            rows = b_self * BS + (m % 2) * 128 + lanes
            cnt_rows[rows] = cnt

            pw = 255.0
            possum = gblk[rows, b_self] - gdiag[rows]
            if paired:
                possum = possum + OTHERWEIGHT * gblk[rows, b_part]
                pw += OTHERWEIGHT * 256.0
            lossnum_rows[rows] = pw * np.log(negsum) - TEMP * possum

            need_refine.extend(rows[mmin < MARGIN])

    # exact recount of borderline rows (replicates reference ops, batched)
    if need_refine:
        idx = np.array(sorted(set(need_refine)), np.int64)
        g_ref = (feats[idx] @ feats.T).astype(np.float32)          # [R, N]
        sim = np.exp(g_ref * np.float32(TEMP)).astype(np.float32)
        for j, i in enumerate(idx):
            negm = labels != labels[i]
            mneg = sim[j, negm].max()
            posm = labels == labels[i]
            posm[i] = False
            cnt_rows[i] = float((sim[j, posm] > mneg).sum())

    acc = np.float32(cnt_rows.sum() / total_pos)
    loss = np.float32(lossnum_rows.sum() / total_pos)
    return acc, loss


# revision 3
# speedup vs baseline: 1.0345x; 1.0161x over previous
"""Trainium2 Bass kernel for nn_ContrastiveLoss (8-core data-parallel).

fp8e4 DoubleRow matmuls (0.5 cyc/row), n-major pipeline:
  ft SBUF [128, 16384] fp8e4: col = k*8192 + q*1024 + s*512 + x,
      value = feats[colperm[q*512+x], k*256 + s*128 + p]
  Per col-tile n (512 cols x 4 m-subtiles): 8 DoubleRow matmuls
      -> PSUM [128, 2048] (ping-pong over n parity); PE p-state warmup MMs.
  Every tile: ACT exp(TEMP*G) -> bf16 e-tile.
  Tiles 1+2, 3+4: DVE pairwise add/max trees + per-m closers -> stats.
  Tiles 5, 6, 0(=both positive-candidate blocks), 7: shipped raw (bf16 e).
Host: negsum/thr/count/margins/possum(exact blocksum trick)/loss assembly;
borderline rows (|G - thr| < MARGIN) recounted exactly in f32.
"""

import os
import sys

sys.path.insert(0, "/opt/trn_rl_repo")
os.environ["BASS_NEVER_TRACE"] = "1"

from contextlib import ExitStack

import numpy as np
import ml_dtypes

import concourse.mybir as mybir
import concourse.tile as tile
from concourse import bacc
from concourse.bass_utils import run_bass_kernel_spmd

TEMP = 0.02
OTHERWEIGHT = 0.5

NCORES = 8
N = 4096
F = 512
BS = 256
NBLK = 16
MTILES = 4
NT = 8            # 512-col tiles
MARGIN = 8.0      # G-domain refinement margin (fp8 matmul noise ~1.2 std)
WARM_MMS = 20     # PE p-state warmup matmuls

F8 = ml_dtypes.float8_e4m3
BF16 = ml_dtypes.bfloat16

_BUILT = None
_LAST_RESULTS = None


def _build_nc():
    f32 = mybir.dt.float32
    bf16 = mybir.dt.bfloat16
    fp8 = mybir.dt.float8e4
    Exp = mybir.ActivationFunctionType.Exp
    DR = mybir.MatmulPerfMode.DoubleRow
    X = mybir.AxisListType.X

    nc = bacc.Bacc("TRN2", target_bir_lowering=False, debug=False)
    ft_d = nc.dram_tensor("ft", [NT, 128, 2048], fp8, kind="ExternalInput")
    # shipped e-tiles (e-domain bf16): slots [n5, n6, n0(tile0), n7]
    eship_d = nc.dram_tensor("eship", [4, 128, 2048], bf16, kind="ExternalOutput")
    # pair stats: [0:4]=sum12 [4:8]=max12 [8:12]=sum34 [12:16]=max34
    stat_d = nc.dram_tensor("stat", [128, 16], bf16, kind="ExternalOutput")

    ADD = mybir.AluOpType.add
    MAX = mybir.AluOpType.max

    with tile.TileContext(nc) as tc, ExitStack() as ctx:
        ftp = ctx.enter_context(tc.tile_pool(name="ft", bufs=1))
        ep = ctx.enter_context(tc.tile_pool(name="e", bufs=7))
        trp = ctx.enter_context(tc.tile_pool(name="tr", bufs=1))
        statp = ctx.enter_context(tc.tile_pool(name="stat", bufs=1))
        psp = ctx.enter_context(tc.tile_pool(name="ps", bufs=1, space="PSUM"))

        ft_t = ftp.tile([128, 16384], fp8, name="ft", tag="ft")
        # view [p, k, q, s, x]
        ftr = ft_t[:].rearrange("p (k q s x) -> p k q s x", k=2, q=NT, s=2)
        ftkr = ft_t[:].rearrange("p (k r) -> p k r", k=2)
        for q in range(NT):
            # one DMA per col-tile: dst [p, k, (s x)=1024], src contiguous
            nc.sync.dma_start(
                ftkr[:, :, q * 1024:(q + 1) * 1024],
                ft_d.ap()[q],
            )

        stat_t = statp.tile([128, 16], bf16, tag="stat")
        # pair partials + halving scratch
        s12 = trp.tile([128, 2048], bf16, name="s12", tag="s12")
        m12 = trp.tile([128, 2048], bf16, name="m12", tag="m12")
        s34 = trp.tile([128, 2048], bf16, name="s34", tag="s34")
        m34 = trp.tile([128, 2048], bf16, name="m34", tag="m34")


        ps_t = [psp.tile([128, 2048], f32, name=f"ps{i}", tag=f"ps{i}") for i in range(2)]

        # PE p-state warmup: keep PE busy from t~0 so real matmuls hit full
        # clock. Warm MMs write a scratch psum region later overwritten.
        warm = ftp.tile([128, 256], fp8, name="warm", tag="warm")
        nc.vector.memset(warm[:], 0.0)
        wr = warm[:].rearrange("p (s x) -> p s x", s=2)
        for _ in range(WARM_MMS):
            nc.tensor.matmul(
                ps_t[0][0:128, 0:128], wr[:, :, 0:128], wr[:, :, 0:128],
                start=True, stop=True, perf_mode=DR,
            )

        def closers(pair_t, col0, op, tag):
            # [128,(4m)(512x)] -> halve twice on DVE 2x -> short 1x reduce
            h1 = trp.tile([128, 1024], bf16, name=f"h1{tag}", tag=f"h1{tag}")
            h2 = trp.tile([128, 512], bf16, name=f"h2{tag}", tag=f"h2{tag}")
            pr = pair_t[:].rearrange("p (m x) -> p m x", m=MTILES)
            ha = h1[:].rearrange("p (m x) -> p m x", m=MTILES)
            hb = h2[:].rearrange("p (m x) -> p m x", m=MTILES)
            with nc.allow_low_precision(reason="bf16 partials; error averages out"):
                nc.vector.tensor_tensor(ha, pr[:, :, 0:256], pr[:, :, 256:512], op=op)
                nc.vector.tensor_tensor(hb, ha[:, :, 0:128], ha[:, :, 128:256], op=op)
                nc.vector.tensor_reduce(
                    stat_t[:, col0:col0 + 4], hb, axis=mybir.AxisListType.X, op=op)

        order = [0, 1, 2, 3, 4, 5, 6, 7]
        ship_slot = {5: 0, 6: 1, 0: 2, 7: 3}
        et_n = {}
        for pos, n in enumerate(order):
            ps = ps_t[pos % 2]
            for m, k in [(m, k) for m in range(MTILES) for k in range(2)]:
                nc.tensor.matmul(
                    ps[:, m * 512:(m + 1) * 512],
                    ftr[:, k, 0, :, m * 128:(m + 1) * 128],   # lhsT [128,2,128]
                    ftr[:, k, n, :, :],                       # rhs  [128,2,512]
                    start=(k == 0),
                    stop=(k == 1),
                    perf_mode=DR,
                )
            et = ep.tile([128, 2048], bf16, name="et", tag="e")
            if n == 7:
                # split the last exp + ship 75/25: shorter final transfer
                nc.scalar.activation(et[:, 0:1536], ps[:, 0:1536], Exp, scale=TEMP)
                nc.sync.dma_start(eship_d.ap()[3, :, 0:1536], et[:, 0:1536])
                nc.scalar.activation(et[:, 1536:2048], ps[:, 1536:2048], Exp, scale=TEMP)
                nc.sync.dma_start(eship_d.ap()[3, :, 1536:2048], et[:, 1536:2048])
            else:
                nc.scalar.activation(et[:], ps[:], Exp, scale=TEMP)
            et_n[n] = et
            with nc.allow_low_precision(reason="bf16 tree; error averages out"):
                if n == 2:
                    nc.vector.tensor_tensor(s12[:], et_n[1][:], et[:], op=ADD)
                    nc.vector.tensor_tensor(m12[:], et_n[1][:], et[:], op=MAX)
                    closers(s12, 0, ADD, "s12")
                    closers(m12, 4, MAX, "m12")
                elif n == 4:
                    nc.vector.tensor_tensor(s34[:], et_n[3][:], et[:], op=ADD)
                    nc.vector.tensor_tensor(m34[:], et_n[3][:], et[:], op=MAX)
                    closers(s34, 8, ADD, "s34")
                    closers(m34, 12, MAX, "m34")
            if n in (0, 5, 6):
                nc.sync.dma_start(eship_d.ap()[ship_slot[n]], et[:])
        nc.sync.dma_start(stat_d.ap()[:], stat_t[:])

    nc.compile()
    return nc


def _labels_np(ov, bs):
    K = ov.shape[0]
    labels1 = np.repeat(np.arange(K), bs)
    non = (ov == 0).astype(np.int64)
    excl = np.cumsum(non) - non
    cls2 = np.where(ov.astype(bool), np.arange(K), K + excl)
    labels2 = np.repeat(cls2, bs)
    return np.concatenate([labels1, labels2])


def kernel(feats1, feats2, overlap_inds, bs):
    global _BUILT, _LAST_RESULTS
    bs = int(bs)
    feats1 = np.asarray(feats1, np.float32)
    feats2 = np.asarray(feats2, np.float32)
    ov = np.asarray(overlap_inds)
    assert feats1.shape == (2048, 512) and feats2.shape == (2048, 512)
    assert bs == BS and ov.shape == (8,)

    feats = np.concatenate([feats1, feats2])              # [N, F]
    labels = _labels_np(ov, bs)                           # [N]
    lblock = labels[::BS]                                 # [16]

    f8 = feats.astype(F8)                                 # fp8-quantized

    in_maps = []
    perms = []
    for c in range(NCORES):
        pb = [c, 8 + c] + [b for b in range(NBLK) if b not in (c, 8 + c)]
        perms.append(pb)
        cols = np.concatenate([np.arange(b * BS, (b + 1) * BS) for b in pb])
        fq = f8[cols]                                     # [4096, 512]
        # [q, x, k, s, p] -> [q, p, k, s, x] -> [8, 128, 2048]
        arr = fq.reshape(NT, 512, 2, 2, 128).transpose(0, 4, 2, 3, 1)
        arr = np.ascontiguousarray(arr.reshape(NT, 128, 2048))
        in_maps.append({"ft": arr})

    if _BUILT is None:
        _BUILT = _build_nc()
    nc = _BUILT

    try:
        res = run_bass_kernel_spmd(nc, in_maps, core_ids=list(range(NCORES)))
    except Exception:
        res = run_bass_kernel_spmd(nc, in_maps, core_ids=list(range(NCORES)))
    _LAST_RESULTS = res

    # ---- host assembly ----
    counts = np.bincount(labels)
    total_pos = float((counts[labels] - 1).sum())

    # exact possum via block sums (f64)
    feats64 = feats.astype(np.float64)
    Sblk = feats64.reshape(NBLK, BS, F).sum(axis=1)       # [16, F]
    gblk = feats64 @ Sblk.T                               # [N, 16] exact
    gdiag = (feats64 * feats64).sum(axis=1)               # [N]

    cnt_rows = np.zeros(N, np.float64)
    lossnum_rows = np.zeros(N, np.float64)
    need_refine = []

    lanes = np.arange(128)
    for c in range(NCORES):
        out = res.results[c]
        stat = out["stat"].astype(np.float64)             # [128, 16] e-domain
        eship = out["eship"].astype(np.float64)           # [4, 128, 2048] e-domain
        pose_e = eship[2]                                 # tile0, e-domain
        with np.errstate(divide="ignore"):
            pose = np.log(pose_e) / TEMP                  # [128, 2048] G-domain
        esh = eship[[0, 1, 3]].reshape(3, 128, 4, 512)    # [ship, p, m, x]
        negsum_big = (stat[:, 0:4] + stat[:, 8:12]
                      + esh.sum(axis=(0, 3)))             # [128, 4]
        bigmax = np.maximum(np.maximum(stat[:, 4:8], stat[:, 12:16]),
                            esh.max(axis=3).max(axis=0))

        for m in range(MTILES):
            b_self = c if m < 2 else 8 + c
            b_part = 8 + c if m < 2 else c
            slot_self = 0 if m < 2 else 1
            paired = lblock[b_self] == lblock[b_part]

            g0 = pose[:, m * 512:(m + 1) * 512]           # [128, 512] tile0
            sl_self = slice(slot_self * 256, slot_self * 256 + 256)
            sl_part = slice((1 - slot_self) * 256, (1 - slot_self) * 256 + 256)
            g_self = g0[:, sl_self]                       # [128, 256]
            g_part = g0[:, sl_part]

            diag_in_self = (m % 2) * 128 + lanes          # col within self block
            e_part = np.exp(TEMP * g_part)

            negsum = negsum_big[:, m].copy()
            thr_e = bigmax[:, m].copy()
            if not paired:
                negsum = negsum + e_part.sum(axis=1)
                thr_e = np.maximum(thr_e, e_part.max(axis=1))
            thr_g = np.log(thr_e) / TEMP

            mask_self = np.ones((128, 256), bool)
            mask_self[lanes, diag_in_self] = False
            cnt = ((g_self > thr_g[:, None]) & mask_self).sum(axis=1).astype(np.float64)
            marg = np.abs(g_self - thr_g[:, None])
            marg[lanes, diag_in_self] = np.inf
            mmin = marg.min(axis=1)
            if paired:
                cnt += (g_part > thr_g[:, None]).sum(axis=1)
                mmin = np.minimum(mmin, np.abs(g_part - thr_g[:, None]).min(axis=1))

            rows = b_self * BS + (m % 2) * 128 + lanes
            cnt_rows[rows] = cnt

            pw = 255.0
            possum = gblk[rows, b_self] - gdiag[rows]
            if paired:
                possum = possum + OTHERWEIGHT * gblk[rows, b_part]
                pw += OTHERWEIGHT * 256.0
            lossnum_rows[rows] = pw * np.log(negsum) - TEMP * possum

            need_refine.extend(rows[mmin < MARGIN])

    # exact recount of borderline rows (replicates reference ops, batched)
    if need_refine:
        idx = np.array(sorted(set(need_refine)), np.int64)
        g_ref = (feats[idx] @ feats.T).astype(np.float32)          # [R, N]
        sim = np.exp(g_ref * np.float32(TEMP)).astype(np.float32)
        for j, i in enumerate(idx):
            negm = labels != labels[i]
            mneg = sim[j, negm].max()
            posm = labels == labels[i]
            posm[i] = False
            cnt_rows[i] = float((sim[j, posm] > mneg).sum())

    acc = np.float32(cnt_rows.sum() / total_pos)
    loss = np.float32(lossnum_rows.sum() / total_pos)
    return acc, loss
